# revision 21
# baseline (speedup 1.0000x reference)
"""Trainium2 Bass kernel for nn_Attention_Layer (B=8, SH=SV=32, DH=D=256, DV=4096).

Math (see reference):
    U_h = h @ U                  (B,SH,D)
    W_v = v @ W                  (B,SV,D)
    f   = tanh(W_v + U_h + b)    (B,SH,SV,D)
    q   = f @ w                  (B,SH,SV,DV)
    e   = exp(q); S = sum_b e; beta = e/S
    u   = sum_sv beta * v        (B,SH,DV)

Sharding: batch-axis normalization (sum over b) makes batch sharding need a
16MB all-reduce; sharding over SH keeps everything core-local.
Each of the 8 cores owns SH/8 = 4 h-positions, all batches. No collectives.

v2 structure (per core):
  - q matmul in fp8e4 with DoubleRow perf mode: fT (tanh output written
    directly as fp8 by ACT) stationary [128,2,128], w (hostside *16, fp8)
    moving [128,2,512] -> halves PE stream cycles; exp un-scales via
    activation scale=1/16.
  - 1024-wide moving operands for the S (identity) and u (indicator L)
    matmuls -> halves instruction + LDWEIGHTS count vs 512-wide.
  - e/g/gv held as single [128, B, 1024] tiles per c-slice; the g and gv
    muls are single wide DVE ops (R broadcast over b via stride-0 AP).
  - last c-slice runs b-granular g/gv so the PE u-matmuls chase the DVE.
"""

import sys

sys.path.insert(0, "/opt/trn_rl_repo")

from contextlib import ExitStack

import ml_dtypes
import numpy as np

import concourse.bass as bass
import concourse.mybir as mybir
import concourse.tile as tile
from concourse import bacc
from concourse.bass_utils import run_bass_kernel_spmd

BF16 = ml_dtypes.bfloat16
F8E4 = ml_dtypes.float8_e4m3
F32 = np.float32

B, SH, SV, DH, DV, D = 8, 32, 32, 256, 4096, 256
NCORES = 8
SHL = SH // NCORES  # 4 h-positions per core
ROWS = B * SHL  # 32 output rows per core, index = b*SHL + h
NT = 4  # number of 1024-wide c' slices
NW = DV // NT  # 1024
W8SCALE = 16.0
WVSCALE = 64.0
KTH = (DV // 128) // 2  # kt-tiles per W/vT precision half


def _ap_key(ap):
    return (
        str(ap.memref),
        ap.offset,
        str(ap.ap),
        str(ap.dtype),
    )


def _dedupe_ldweights(nc):
    """Drop InstLdweights whose stationary matches the previous PE weight
    load (bass splits every matmul into Ldweights+Matmult at emit time, so
    back-to-back matmuls sharing a stationary reload it needlessly).
    A redundant Ldweights carrying a semaphore wait has the wait moved onto
    the following matmul when that matmul has none; otherwise it's kept."""
    n_drop = n_movewait = 0
    for fn in nc.m.functions:
        for blk in fn.blocks:
            insts = list(blk.instructions)
            keep = []
            last_key = None
            drop_wait = None
            for idx, inst in enumerate(insts):
                nm = type(inst).__name__
                if getattr(inst, "engine", None) != mybir.EngineType.PE:
                    keep.append(inst)
                    continue
                if nm == "InstLdweights":
                    key = (_ap_key(inst.ins[0]), str(inst.perf_mode))
                    si = inst.sync_info
                    has_wait = si is not None and len(si.on_wait) > 0
                    has_upd = si is not None and len(si.on_update) > 0
                    if key == last_key and not has_upd:
                        if not has_wait:
                            n_drop += 1
                            continue
                        # find next PE matmul; move the wait if it has none
                        nxt = None
                        for j in range(idx + 1, len(insts)):
                            if (
                                type(insts[j]).__name__ == "InstMatmult"
                                and insts[j].engine == mybir.EngineType.PE
                            ):
                                nxt = insts[j]
                                break
                        if nxt is not None and (
                            nxt.sync_info is None
                            or len(nxt.sync_info.on_wait) == 0
                        ):
                            if nxt.sync_info is None:
                                nxt.sync_info = mybir.SyncInfo(
                                    on_wait=list(si.on_wait), on_update=[]
                                )
                            else:
                                nxt.sync_info.on_wait = list(si.on_wait)
                            n_drop += 1
                            n_movewait += 1
                            continue
                        keep.append(inst)
                        continue
                    last_key = key
                    keep.append(inst)
                    continue
                if nm in ("InstMatmult", "InstEventSemaphore", "InstNop"):
                    keep.append(inst)
                    continue
                last_key = None  # unknown PE op: conservatively reset
                keep.append(inst)
            if len(keep) != len(insts):
                try:
                    blk.instructions[:] = keep
                except TypeError:
                    blk.instructions.clear()
                    blk.instructions.extend(keep)
    return n_drop, n_movewait


def build_nc(debug: bool = False):
    nc = bacc.Bacc("TRN2", target_bir_lowering=False, debug=debug)
    f32, bf, f8 = mybir.dt.float32, mybir.dt.bfloat16, mybir.dt.float8e4

    # All inputs are pre-laid-out on the host partition-major so every DMA
    # reads long contiguous runs per partition (512B descriptors otherwise
    # saturate the DMA engines with per-descriptor overhead).
    W_d = nc.dram_tensor("Wp", (128, KTH, D), bf, kind="ExternalInput")
    W8_d = nc.dram_tensor("Wp8", (128, KTH, D), f8, kind="ExternalInput")
    vT_d = nc.dram_tensor("vTp", (128, KTH, B * SV), bf, kind="ExternalInput")
    vT8_d = nc.dram_tensor("vTp8", (128, KTH, B * SV), f8, kind="ExternalInput")
    w8_d = nc.dram_tensor("w8p", (128, D // 128, DV), f8, kind="ExternalInput")
    v3_d = nc.dram_tensor("v3q", (NT, 128, B, NW), bf, kind="ExternalInput")
    U2_d = nc.dram_tensor("U2", (DH + 1, D), f32, kind="ExternalInput")
    hT2_d = nc.dram_tensor("hT2", (DH + 1, ROWS), f32, kind="ExternalInput")
    I_d = nc.dram_tensor("Ieye", (128, 128), bf, kind="ExternalInput")
    L_d = nc.dram_tensor("Lsump", (128, B, ROWS), bf, kind="ExternalInput")
    u_d = nc.dram_tensor("u_out", (ROWS, DV), f32, kind="ExternalOutput")

    KT_C = DV // 128  # 32 k-tiles over the DV contraction (v @ W)
    KT_D = D // 128  # 2 k-tiles over the D contraction (f @ w)

    with tile.TileContext(nc) as tc, ExitStack() as ctx:
        consts = ctx.enter_context(tc.tile_pool(name="consts", bufs=1))

        ph1_ctx = ExitStack()
        ph1c = ph1_ctx.enter_context(tc.tile_pool(name="ph1c", bufs=1))

        # ---- resident constants -------------------------------------------
        U2_sb = ph1c.tile([128, 3, D], f32)
        nc.gpsimd.dma_start(out=U2_sb[:, 0, :], in_=U2_d[0:128, :])
        nc.gpsimd.dma_start(out=U2_sb[:, 1, :], in_=U2_d[128:256, :])
        nc.gpsimd.dma_start(out=U2_sb[0:1, 2, :], in_=U2_d[256:257, :])
        hT2_sb = ph1c.tile([128, 3, ROWS], f32)
        nc.gpsimd.dma_start(out=hT2_sb[:, 0, :], in_=hT2_d[0:128, :])
        nc.gpsimd.dma_start(out=hT2_sb[:, 1, :], in_=hT2_d[128:256, :])
        nc.gpsimd.dma_start(out=hT2_sb[0:1, 2, :], in_=hT2_d[256:257, :])

        # phase-1-only weights in ph1c (freed before the big loop); W/vT
        # interleaved in 8 chunks so the Wv k-loop chases the DMA.
        NCH = 8
        CH = KT_C // NCH
        W_sb = ph1c.tile([128, KTH, D], bf)
        W8_sbt = ph1c.tile([128, KTH, D], f8)
        vT_sb = ph1c.tile([128, KTH, B * SV], bf)
        vT8_sb = ph1c.tile([128, KTH, B * SV], f8)
        for ch in range(NCH):
            sl = slice(ch * CH, (ch + 1) * CH)
            if ch < NCH // 2:
                nc.sync.dma_start(out=W_sb[:, sl, :], in_=W_d[:, sl, :])
                nc.gpsimd.dma_start(out=vT_sb[:, sl, :], in_=vT_d[:, sl, :])
            else:
                s8 = slice((ch - NCH // 2) * CH, (ch - NCH // 2 + 1) * CH)
                nc.sync.dma_start(out=W8_sbt[:, s8, :], in_=W8_d[:, s8, :])
                nc.gpsimd.dma_start(out=vT8_sb[:, s8, :], in_=vT8_d[:, s8, :])
        w8_sb = consts.tile([128, KT_D, DV], f8)
        nc.sync.dma_start(out=w8_sb, in_=w8_d[:])
        I_sb = consts.tile([128, 128], bf)
        nc.gpsimd.dma_start(out=I_sb, in_=I_d[:])
        L_sb = consts.tile([128, B, ROWS], bf)
        nc.gpsimd.dma_start(out=L_sb, in_=L_d[:])

        # v pre-replicated on the host over the 4 local h positions and
        # laid out c-quarter-major: 4 fat contiguous DMAs, after the
        # phase-1-critical W/vT loads on the same queue.
        v_rep = consts.tile([128, NT, B, NW], bf)
        for cq in range(NT):
            nc.sync.dma_start(out=v_rep[:, cq, :, :], in_=v3_d[cq])

        # ---- phase 1: ubias = U^T h + bias, W_v^T, fT = tanh(...) ---------
        ph1 = ph1_ctx.enter_context(tc.tile_pool(name="ph1", bufs=1, space="PSUM"))

        ub_ps = ph1.tile([128, 2, ROWS], f32)
        for mt in range(2):
            msl = slice(mt * 128, (mt + 1) * 128)
            for kt in range(3):
                ksz = 128 if kt < 2 else 1
                nc.tensor.matmul(
                    ub_ps[:, mt, :],
                    U2_sb[0:ksz, kt, msl],
                    hT2_sb[0:ksz, kt, :],
                    start=(kt == 0),
                    stop=(kt == 2),
                )
        ub_sb = ph1c.tile([128, 2, ROWS], f32)
        nc.vector.tensor_copy(ub_sb, ub_ps)

        wv_ps = [
            ph1.tile([128, B * SV], f32, tag=f"wv{mt}", name=f"wv_ps{mt}")
            for mt in range(2)
        ]
        for kt in range(KT_C):  # kt-major so chunk 0 starts while chunk 1 loads
            lo = kt < KTH
            Wt = W_sb if lo else W8_sbt
            vt = vT_sb if lo else vT8_sb
            ki = kt if lo else kt - KTH
            for mt in range(2):
                msl = slice(mt * 128, (mt + 1) * 128)
                nc.tensor.matmul(
                    wv_ps[mt],
                    Wt[:, ki, msl],
                    vt[:, ki, :],
                    start=(kt == 0),
                    stop=(kt == KT_C - 1),
                )

        # zz[d, (b,h,s)] = W_v^T[d, (b,s)] + ubias[d, (b,h)], fT8 = tanh(zz)
        zz_sb = ph1c.tile([128, 2, B * SHL * SV], f32)
        fT8_sb = consts.tile([128, KT_D, B * SHL * SV], f8)
        for mt in range(2):
            wv_base = wv_ps[mt][:]
            wv_bc = bass.AP(
                tensor=wv_base.tensor,
                offset=wv_base.offset,
                ap=[wv_base.ap[0], [32, B], [0, SHL], [1, SV]],
            )
            ub_base = ub_sb[:, mt, :]
            ub_bc = bass.AP(
                tensor=ub_base.tensor,
                offset=ub_base.offset,
                ap=[ub_base.ap[0], [SHL, B], [1, SHL], [0, SV]],
            )
            zz_out = zz_sb[:, mt, :].rearrange("p (b h s) -> p b h s", b=B, h=SHL)
            nc.vector.tensor_add(zz_out, wv_bc, ub_bc)
        for bh in range(4):  # slice-major: q(b0) can start after 2 tanhs
            bsl = slice(bh * 256, (bh + 1) * 256)
            for mt in range(2):
                nc.scalar.activation(
                    fT8_sb[:, mt, bsl],
                    zz_sb[:, mt, bsl],
                    mybir.ActivationFunctionType.Tanh,
                    scale=1.0 / WVSCALE,
                )

        ph1_ctx.close()

        # ---- phase 2: q -> e -> S -> R -> g -> gv -> u --------------------
        epool = ctx.enter_context(tc.tile_pool(name="epool", bufs=2))
        gpool = ctx.enter_context(tc.tile_pool(name="gpool", bufs=2))
        gvpool = ctx.enter_context(tc.tile_pool(name="gvpool", bufs=3))
        usbpool = ctx.enter_context(tc.tile_pool(name="usbpool", bufs=2))
        r32pool = ctx.enter_context(tc.tile_pool(name="r32pool", bufs=2))
        rpool = ctx.enter_context(tc.tile_pool(name="rpool", bufs=2))
        qpool = ctx.enter_context(tc.tile_pool(name="qpool", bufs=2, space="PSUM"))
        spool = ctx.enter_context(tc.tile_pool(name="spool", bufs=1, space="PSUM"))
        upool = ctx.enter_context(tc.tile_pool(name="upool", bufs=1, space="PSUM"))

        # Emit consecutive matmuls that share a stationary with
        # ldweights=False on all but the first so codegen skips the reload.
        SKIP_LDW = False

        def skip_ldw(mm):
            if SKIP_LDW:
                mm.ins.ldweights = False

        pending = []  # deferred u-matmul blocks, two prs late

        def emit_u(gv_t, csl):
            u_ps = upool.tile([32, NW], f32, tag="u")
            for bb in range(B):
                for half in range(2):
                    usl = slice(half * 512, (half + 1) * 512)
                    nc.tensor.matmul(
                        u_ps[:, usl],
                        L_sb[:, bb, :],
                        gv_t[:, bb, usl],
                        start=(bb == 0),
                        stop=(bb == B - 1),
                        skip_group_check=True,
                    )
            u_sb = usbpool.tile([32, NW], f32, tag="u_sb")
            nc.scalar.copy(u_sb[:, 0:512], u_ps[:, 0:512])
            nc.vector.tensor_copy(u_sb[:, 512:1024], u_ps[:, 512:1024])
            nc.sync.dma_start(out=u_d[:, csl], in_=u_sb)

        for pr in range(NT):
            csl = slice(pr * NW, (pr + 1) * NW)
            last = pr == NT - 1

            e_t = epool.tile([128, B, NW], bf, tag="e")
            for bb in range(B):
                q_ps = qpool.tile([128, NW], f32, tag="q", name=f"q_{pr}_{bb}")
                for half in range(2):
                    hsl = slice(pr * NW + half * 512, pr * NW + (half + 1) * 512)
                    mm = nc.tensor.matmul(
                        q_ps[:, half * 512 : (half + 1) * 512],
                        fT8_sb[:, :, bb * 128 : (bb + 1) * 128],
                        w8_sb[:, :, hsl],
                        start=True,
                        stop=True,
                        perf_mode=mybir.MatmulPerfMode.DoubleRow,
                        skip_group_check=True,
                    )
                    if half == 1:
                        skip_ldw(mm)
                nc.scalar.activation(
                    e_t[:, bb, :],
                    q_ps,
                    mybir.ActivationFunctionType.Exp,
                    scale=1.0 / W8SCALE,
                )

            # S-matmuls batched after the b-loop: identity stationary is
            # shared, only the first load is real
            s_ps = spool.tile([128, NW], f32, tag="s")
            first = True
            for half in range(2):
                ssl = slice(half * 512, (half + 1) * 512)
                for bb in range(B):
                    mm = nc.tensor.matmul(
                        s_ps[:, ssl],
                        I_sb,
                        e_t[:, bb, ssl],
                        start=(bb == 0),
                        stop=(bb == B - 1),
                        skip_group_check=True,
                    )
                    if not first:
                        skip_ldw(mm)
                    first = False

            if last:
                # drain all deferred u-blocks first: PE has work while the
                # DVE runs the tail normalization chain
                while pending:
                    emit_u(*pending.pop(0))
            elif len(pending) > 1:
                emit_u(*pending.pop(0))

            # R = 1/S: custom-DVE approx reciprocal writing bf16 directly
            # (the fp32-only assert in the wrapper is about the seed's input
            # bit layout; the output write-port cast to bf16 is fine)
            from concourse.dve_ops import (
                RECIP_APPROX_FAST_CONSTS as _RC,
                RECIPROCAL_APPROX_FAST as _RAF,
            )

            r_bf = rpool.tile([128, NW], bf, tag="r")
            nc.vector._custom_dve(
                _RAF, out=r_bf, in0=s_ps, s0=_RC["s0"], s1=_RC["s1"], imm2=_RC["imm2"]
            )

            r_base = r_bf[:]
            r_bc = bass.AP(
                tensor=r_base.tensor,
                offset=r_base.offset,
                ap=[r_base.ap[0], [0, B], [1, NW]],
            )

            g_t = gpool.tile([128, B, NW], bf, tag="g")
            gv_t = gvpool.tile([128, B, NW], bf, tag="gv")
            if not last:
                r2 = bass.AP(
                    tensor=r_base.tensor,
                    offset=r_base.offset,
                    ap=[r_base.ap[0], [0, 2], [1, NW]],
                )
                for bp in range(B // 2):
                    bsl = slice(2 * bp, 2 * bp + 2)
                    nc.vector.tensor_mul(g_t[:, bsl, :], e_t[:, bsl, :], r2)
                    nc.vector.tensor_mul(
                        gv_t[:, bsl, :], g_t[:, bsl, :], v_rep[:, pr, bsl, :]
                    )
                pending.append((gv_t, csl))
            else:
                # b-pair granular so the PE u-matmuls chase the DVE chain
                r2 = bass.AP(
                    tensor=r_base.tensor,
                    offset=r_base.offset,
                    ap=[r_base.ap[0], [0, 2], [1, NW]],
                )
                u_ps = upool.tile([32, NW], f32, tag="u")
                for bp in range(B // 2):
                    bsl = slice(2 * bp, 2 * bp + 2)
                    nc.vector.tensor_mul(g_t[:, bsl, :], e_t[:, bsl, :], r2)
                    nc.vector.tensor_mul(
                        gv_t[:, bsl, :], g_t[:, bsl, :], v_rep[:, pr, bsl, :]
                    )
                    for bb in range(2 * bp, 2 * bp + 2):
                        for half in range(2):
                            usl = slice(half * 512, (half + 1) * 512)
                            nc.tensor.matmul(
                                u_ps[:, usl],
                                L_sb[:, bb, :],
                                gv_t[:, bb, usl],
                                start=(bb == 0),
                                stop=(bb == B - 1),
                                skip_group_check=True,
                            )
                u_sb = usbpool.tile([32, NW], f32, tag="u_sb")
                nc.scalar.copy(u_sb, u_ps)
                nc.sync.dma_start(out=u_d[:, csl], in_=u_sb)

        for p in pending:
            emit_u(*p)

    nc.compile()
    nd, nw = _dedupe_ldweights(nc)
    print(f"ldweights dedupe: dropped {nd} (moved {nw} waits)")
    return nc


def _install_profile_hook():
    """The image's antenv lacks axon_hooks; inject it and register the
    ctypes NTFF hook from trn_agent_boot so trace=True works under axon."""
    import types

    try:
        from antenv.axon_hooks import get_axon_ntff_profile_hook  # noqa: F401

        return
    except ImportError:
        pass
    import antenv

    mod = types.ModuleType("antenv.axon_hooks")
    holder = {"hook": None}
    mod.set_axon_ntff_profile_hook = lambda h: holder.__setitem__("hook", h)
    mod.get_axon_ntff_profile_hook = lambda: holder["hook"]
    sys.modules["antenv.axon_hooks"] = mod
    antenv.axon_hooks = mod
    try:
        if "/root/.axon_site" not in sys.path:
            sys.path.insert(0, "/root/.axon_site")
        from trn_agent_boot.trn_boot import _ntff_profile_via_ctypes

        mod.set_axon_ntff_profile_hook(
            _ntff_profile_via_ctypes("/opt/axon/libaxon_pjrt.so")
        )
    except Exception as ex:  # degrade: tracing skipped, run still works
        print("profile hook install failed:", ex)
    # artifact upload needs bucket creds this container doesn't have
    import concourse.bass_utils as bu

    bu.upload_artifacts = lambda tmpdir: "local://" + tmpdir


_NC_CACHE = {}


def _get_nc():
    if "nc" not in _NC_CACHE:
        _NC_CACHE["nc"] = build_nc()
    return _NC_CACHE["nc"]


def make_inputs(h, v, W, U, b, w):
    """Host-side prep: shared tensors + per-core in_maps."""
    Wk = (W * WVSCALE).reshape(DV // 128, 128, D).transpose(1, 0, 2)
    Wp = np.ascontiguousarray(Wk[:, :KTH]).astype(BF16)
    Wp8 = np.ascontiguousarray(Wk[:, KTH:]).astype(F8E4)
    vT = v.transpose(2, 0, 1).reshape(DV, B * SV)
    vTk = vT.reshape(DV // 128, 128, B * SV).transpose(1, 0, 2)
    vTp = np.ascontiguousarray(vTk[:, :KTH]).astype(BF16)
    vTp8 = np.ascontiguousarray(vTk[:, KTH:]).astype(F8E4)
    w8p = np.ascontiguousarray(
        (w * W8SCALE).reshape(D // 128, 128, DV).transpose(1, 0, 2)
    ).astype(F8E4)
    vq = v.transpose(1, 0, 2).reshape(SV, B, NT, NW)  # (s, b, cq, cw)
    v3q = np.ascontiguousarray(
        np.tile(vq.transpose(2, 0, 1, 3), (1, SHL, 1, 1))
    ).astype(BF16)  # (cq, 128=(h,s), b, cw)
    U2 = np.concatenate([U * WVSCALE, b[None, :] * WVSCALE], axis=0).astype(F32)
    Ieye = np.eye(128, dtype=BF16)
    Lsum = np.zeros((128, B, ROWS), dtype=BF16)
    for bb in range(B):
        for hh in range(SHL):
            for ss in range(SV):
                Lsum[hh * SV + ss, bb, bb * SHL + hh] = 1
    in_maps = []
    for core in range(NCORES):
        hsl = h[:, core * SHL : (core + 1) * SHL, :]  # (B, SHL, DH)
        hT = np.ascontiguousarray(hsl.transpose(2, 0, 1).reshape(DH, ROWS))
        hT2 = np.concatenate([hT, np.ones((1, ROWS), F32)], axis=0).astype(F32)
        in_maps.append(
            {
                "Wp": Wp,
                "Wp8": Wp8,
                "vTp": vTp,
                "vTp8": vTp8,
                "w8p": w8p,
                "v3q": v3q,
                "U2": U2,
                "hT2": hT2,
                "Ieye": Ieye,
                "Lsump": Lsum,
            }
        )
    return in_maps


def gather_output(results):
    u_full = np.empty((B, SH, DV), dtype=F32)
    for core, res in enumerate(results):
        u_full[:, core * SHL : (core + 1) * SHL, :] = res["u_out"].reshape(
            B, SHL, DV
        )
    return u_full


def kernel(h, v, W, U, b, w, trace: bool = False):
    if trace:
        _install_profile_hook()
    nc = _get_nc()
    in_maps = make_inputs(
        np.asarray(h, F32),
        np.asarray(v, F32),
        np.asarray(W, F32),
        np.asarray(U, F32),
        np.asarray(b, F32),
        np.asarray(w, F32),
    )
    out = run_bass_kernel_spmd(nc, in_maps, core_ids=list(range(NCORES)), trace=trace)
    res = gather_output(out.results)
    if trace:
        kernel.last_exec_time_ns = out.exec_time_ns
        kernel.last_trace = out.instructions_and_trace
    return res


# revision 22
# speedup vs baseline: 1.1029x; 1.1029x over previous
"""Trainium2 Bass kernel for nn_Attention_Layer (B=8, SH=SV=32, DH=D=256, DV=4096).

Math (see reference):
    U_h = h @ U                  (B,SH,D)
    W_v = v @ W                  (B,SV,D)
    f   = tanh(W_v + U_h + b)    (B,SH,SV,D)
    q   = f @ w                  (B,SH,SV,DV)
    e   = exp(q); S = sum_b e; beta = e/S
    u   = sum_sv beta * v        (B,SH,DV)

Sharding: batch-axis normalization (sum over b) makes batch sharding need a
16MB all-reduce; sharding over SH keeps everything core-local.
Each of the 8 cores owns SH/8 = 4 h-positions, all batches. No collectives.

v2 structure (per core):
  - q matmul in fp8e4 with DoubleRow perf mode: fT (tanh output written
    directly as fp8 by ACT) stationary [128,2,128], w (hostside *16, fp8)
    moving [128,2,512] -> halves PE stream cycles; exp un-scales via
    activation scale=1/16.
  - 1024-wide moving operands for the S (identity) and u (indicator L)
    matmuls -> halves instruction + LDWEIGHTS count vs 512-wide.
  - e/g/gv held as single [128, B, 1024] tiles per c-slice; the g and gv
    muls are single wide DVE ops (R broadcast over b via stride-0 AP).
  - last c-slice runs b-granular g/gv so the PE u-matmuls chase the DVE.
"""

import sys

sys.path.insert(0, "/opt/trn_rl_repo")

from contextlib import ExitStack

import ml_dtypes
import numpy as np

import concourse.bass as bass
import concourse.mybir as mybir
import concourse.tile as tile
from concourse import bacc
from concourse.bass_utils import run_bass_kernel_spmd

BF16 = ml_dtypes.bfloat16
F8E4 = ml_dtypes.float8_e4m3
F32 = np.float32

B, SH, SV, DH, DV, D = 8, 32, 32, 256, 4096, 256
NCORES = 8
SHL = SH // NCORES  # 4 h-positions per core
ROWS = B * SHL  # 32 output rows per core, index = b*SHL + h
NT = 4  # number of 1024-wide c' slices
NW = DV // NT  # 1024
W8SCALE = 16.0
WVSCALE = 64.0
KTH = (DV // 128) // 2  # kt-tiles per W/vT precision half


def _ap_key(ap):
    return (
        str(ap.memref),
        ap.offset,
        str(ap.ap),
        str(ap.dtype),
    )


def _dedupe_ldweights(nc):
    """Drop InstLdweights whose stationary matches the previous PE weight
    load (bass splits every matmul into Ldweights+Matmult at emit time, so
    back-to-back matmuls sharing a stationary reload it needlessly).
    A redundant Ldweights carrying a semaphore wait has the wait moved onto
    the following matmul when that matmul has none; otherwise it's kept."""
    n_drop = n_movewait = 0
    for fn in nc.m.functions:
        for blk in fn.blocks:
            insts = list(blk.instructions)
            keep = []
            last_key = None
            drop_wait = None
            for idx, inst in enumerate(insts):
                nm = type(inst).__name__
                if getattr(inst, "engine", None) != mybir.EngineType.PE:
                    keep.append(inst)
                    continue
                if nm == "InstLdweights":
                    key = (_ap_key(inst.ins[0]), str(inst.perf_mode))
                    si = inst.sync_info
                    has_wait = si is not None and len(si.on_wait) > 0
                    has_upd = si is not None and len(si.on_update) > 0
                    if key == last_key and not has_upd:
                        if not has_wait:
                            n_drop += 1
                            continue
                        # find next PE matmul; move the wait if it has none
                        nxt = None
                        for j in range(idx + 1, len(insts)):
                            if (
                                type(insts[j]).__name__ == "InstMatmult"
                                and insts[j].engine == mybir.EngineType.PE
                            ):
                                nxt = insts[j]
                                break
                        if nxt is not None and (
                            nxt.sync_info is None
                            or len(nxt.sync_info.on_wait) == 0
                        ):
                            if nxt.sync_info is None:
                                nxt.sync_info = mybir.SyncInfo(
                                    on_wait=list(si.on_wait), on_update=[]
                                )
                            else:
                                nxt.sync_info.on_wait = list(si.on_wait)
                            n_drop += 1
                            n_movewait += 1
                            continue
                        keep.append(inst)
                        continue
                    last_key = key
                    keep.append(inst)
                    continue
                if nm in ("InstMatmult", "InstEventSemaphore", "InstNop"):
                    keep.append(inst)
                    continue
                last_key = None  # unknown PE op: conservatively reset
                keep.append(inst)
            if len(keep) != len(insts):
                try:
                    blk.instructions[:] = keep
                except TypeError:
                    blk.instructions.clear()
                    blk.instructions.extend(keep)
    return n_drop, n_movewait


def build_nc(debug: bool = False):
    nc = bacc.Bacc("TRN2", target_bir_lowering=False, debug=debug)
    f32, bf, f8 = mybir.dt.float32, mybir.dt.bfloat16, mybir.dt.float8e4

    # All inputs are pre-laid-out on the host partition-major so every DMA
    # reads long contiguous runs per partition (512B descriptors otherwise
    # saturate the DMA engines with per-descriptor overhead).
    W_d = nc.dram_tensor("Wp", (128, DV // 128, D), bf, kind="ExternalInput")
    vT_d = nc.dram_tensor("vTp", (128, DV // 128, B * SV), bf, kind="ExternalInput")
    w8_d = nc.dram_tensor("w8p", (128, D // 128, DV), f8, kind="ExternalInput")
    v3_d = nc.dram_tensor("v3q", (NT, 128, B, NW), bf, kind="ExternalInput")
    U2_d = nc.dram_tensor("U2", (DH + 1, D), f32, kind="ExternalInput")
    hT2_d = nc.dram_tensor("hT2", (DH + 1, ROWS), f32, kind="ExternalInput")
    I_d = nc.dram_tensor("Ieye", (128, 128), bf, kind="ExternalInput")
    L_d = nc.dram_tensor("Lsump", (128, B, ROWS), bf, kind="ExternalInput")
    u_d = nc.dram_tensor("u_out", (ROWS, DV), f32, kind="ExternalOutput")

    KT_C = DV // 128  # 32 k-tiles over the DV contraction (v @ W)
    KT_D = D // 128  # 2 k-tiles over the D contraction (f @ w)

    with tile.TileContext(nc) as tc, ExitStack() as ctx:
        consts = ctx.enter_context(tc.tile_pool(name="consts", bufs=1))

        ph1_ctx = ExitStack()
        ph1c = ph1_ctx.enter_context(tc.tile_pool(name="ph1c", bufs=1))

        # ---- resident constants -------------------------------------------
        U2_sb = ph1c.tile([128, 3, D], f32)
        nc.gpsimd.dma_start(out=U2_sb[:, 0, :], in_=U2_d[0:128, :])
        nc.gpsimd.dma_start(out=U2_sb[:, 1, :], in_=U2_d[128:256, :])
        nc.gpsimd.dma_start(out=U2_sb[0:1, 2, :], in_=U2_d[256:257, :])
        hT2_sb = ph1c.tile([128, 3, ROWS], f32)
        nc.gpsimd.dma_start(out=hT2_sb[:, 0, :], in_=hT2_d[0:128, :])
        nc.gpsimd.dma_start(out=hT2_sb[:, 1, :], in_=hT2_d[128:256, :])
        nc.gpsimd.dma_start(out=hT2_sb[0:1, 2, :], in_=hT2_d[256:257, :])

        # phase-1-only weights in ph1c (freed before the big loop); W/vT
        # interleaved in 8 chunks so the Wv k-loop chases the DMA.
        NCH = 8
        CH = KT_C // NCH
        W_sb = ph1c.tile([128, KT_C, D], bf)
        vT_sb = ph1c.tile([128, KT_C, B * SV], bf)
        for ch in range(NCH):
            sl = slice(ch * CH, (ch + 1) * CH)
            nc.sync.dma_start(out=W_sb[:, sl, :], in_=W_d[:, sl, :])
            nc.sync.dma_start(out=vT_sb[:, sl, :], in_=vT_d[:, sl, :])
        w8_sb = consts.tile([128, KT_D, DV], f8)
        nc.sync.dma_start(out=w8_sb, in_=w8_d[:])
        I_sb = consts.tile([128, 128], bf)
        nc.gpsimd.dma_start(out=I_sb, in_=I_d[:])
        L_sb = consts.tile([128, B, ROWS], bf)
        nc.gpsimd.dma_start(out=L_sb, in_=L_d[:])

        # v pre-replicated on the host over the 4 local h positions and
        # laid out c-quarter-major: 4 fat contiguous DMAs, after the
        # phase-1-critical W/vT loads on the same queue.
        v_rep = consts.tile([128, NT, B, NW], bf)
        for cq in range(NT):
            nc.sync.dma_start(out=v_rep[:, cq, :, :], in_=v3_d[cq])

        # ---- phase 1: ubias = U^T h + bias, W_v^T, fT = tanh(...) ---------
        ph1 = ph1_ctx.enter_context(tc.tile_pool(name="ph1", bufs=1, space="PSUM"))

        ub_ps = ph1.tile([128, 2, ROWS], f32)
        for mt in range(2):
            msl = slice(mt * 128, (mt + 1) * 128)
            for kt in range(3):
                ksz = 128 if kt < 2 else 1
                nc.tensor.matmul(
                    ub_ps[:, mt, :],
                    U2_sb[0:ksz, kt, msl],
                    hT2_sb[0:ksz, kt, :],
                    start=(kt == 0),
                    stop=(kt == 2),
                )
        ub_sb = ph1c.tile([128, 2, ROWS], f32)
        nc.vector.tensor_copy(ub_sb, ub_ps)

        wv_ps = [
            ph1.tile([128, B * SV], f32, tag=f"wv{mt}", name=f"wv_ps{mt}")
            for mt in range(2)
        ]
        for kt in range(KT_C):  # kt-major so chunk 0 starts while chunk 1 loads
            for mt in range(2):
                msl = slice(mt * 128, (mt + 1) * 128)
                nc.tensor.matmul(
                    wv_ps[mt],
                    W_sb[:, kt, msl],
                    vT_sb[:, kt, :],
                    start=(kt == 0),
                    stop=(kt == KT_C - 1),
                )

        # zz[d, (b,h,s)] = W_v^T[d, (b,s)] + ubias[d, (b,h)], fT8 = tanh(zz)
        zz_sb = ph1c.tile([128, 2, B * SHL * SV], f32)
        fT8_sb = consts.tile([128, KT_D, B * SHL * SV], f8)
        for mt in range(2):
            wv_base = wv_ps[mt][:]
            wv_bc = bass.AP(
                tensor=wv_base.tensor,
                offset=wv_base.offset,
                ap=[wv_base.ap[0], [32, B], [0, SHL], [1, SV]],
            )
            ub_base = ub_sb[:, mt, :]
            ub_bc = bass.AP(
                tensor=ub_base.tensor,
                offset=ub_base.offset,
                ap=[ub_base.ap[0], [SHL, B], [1, SHL], [0, SV]],
            )
            zz_out = zz_sb[:, mt, :].rearrange("p (b h s) -> p b h s", b=B, h=SHL)
            nc.vector.tensor_add(zz_out, wv_bc, ub_bc)
        for bh in range(4):  # slice-major: q(b0) can start after 2 tanhs
            bsl = slice(bh * 256, (bh + 1) * 256)
            for mt in range(2):
                nc.scalar.activation(
                    fT8_sb[:, mt, bsl],
                    zz_sb[:, mt, bsl],
                    mybir.ActivationFunctionType.Tanh,
                )

        ph1_ctx.close()

        # ---- phase 2: q -> e -> S -> R -> g -> gv -> u --------------------
        epool = ctx.enter_context(tc.tile_pool(name="epool", bufs=2))
        gpool = ctx.enter_context(tc.tile_pool(name="gpool", bufs=2))
        gvpool = ctx.enter_context(tc.tile_pool(name="gvpool", bufs=3))
        usbpool = ctx.enter_context(tc.tile_pool(name="usbpool", bufs=2))
        r32pool = ctx.enter_context(tc.tile_pool(name="r32pool", bufs=2))
        rpool = ctx.enter_context(tc.tile_pool(name="rpool", bufs=2))
        qpool = ctx.enter_context(tc.tile_pool(name="qpool", bufs=2, space="PSUM"))
        spool = ctx.enter_context(tc.tile_pool(name="spool", bufs=1, space="PSUM"))
        upool = ctx.enter_context(tc.tile_pool(name="upool", bufs=1, space="PSUM"))

        # Emit consecutive matmuls that share a stationary with
        # ldweights=False on all but the first so codegen skips the reload.
        SKIP_LDW = False

        def skip_ldw(mm):
            if SKIP_LDW:
                mm.ins.ldweights = False

        pending = []  # deferred u-matmul blocks, two prs late

        def emit_u(gv_t, csl):
            u_ps = upool.tile([32, NW], f32, tag="u")
            for bb in range(B):
                for half in range(2):
                    usl = slice(half * 512, (half + 1) * 512)
                    nc.tensor.matmul(
                        u_ps[:, usl],
                        L_sb[:, bb, :],
                        gv_t[:, bb, usl],
                        start=(bb == 0),
                        stop=(bb == B - 1),
                        skip_group_check=True,
                    )
            u_sb = usbpool.tile([32, NW], f32, tag="u_sb")
            nc.scalar.copy(u_sb, u_ps)
            nc.sync.dma_start(out=u_d[:, csl], in_=u_sb)

        for pr in range(NT):
            csl = slice(pr * NW, (pr + 1) * NW)
            last = pr == NT - 1

            e_t = epool.tile([128, B, NW], bf, tag="e")
            for bb in range(B):
                q_ps = qpool.tile([128, NW], f32, tag="q", name=f"q_{pr}_{bb}")
                for half in range(2):
                    hsl = slice(pr * NW + half * 512, pr * NW + (half + 1) * 512)
                    mm = nc.tensor.matmul(
                        q_ps[:, half * 512 : (half + 1) * 512],
                        fT8_sb[:, :, bb * 128 : (bb + 1) * 128],
                        w8_sb[:, :, hsl],
                        start=True,
                        stop=True,
                        perf_mode=mybir.MatmulPerfMode.DoubleRow,
                        skip_group_check=True,
                    )
                    if half == 1:
                        skip_ldw(mm)
                nc.scalar.activation(
                    e_t[:, bb, :],
                    q_ps,
                    mybir.ActivationFunctionType.Exp,
                    scale=1.0 / W8SCALE,
                )

            # S-matmuls batched after the b-loop: identity stationary is
            # shared, only the first load is real
            s_ps = spool.tile([128, NW], f32, tag="s")
            first = True
            for half in range(2):
                ssl = slice(half * 512, (half + 1) * 512)
                for bb in range(B):
                    mm = nc.tensor.matmul(
                        s_ps[:, ssl],
                        I_sb,
                        e_t[:, bb, ssl],
                        start=(bb == 0),
                        stop=(bb == B - 1),
                        skip_group_check=True,
                    )
                    if not first:
                        skip_ldw(mm)
                    first = False

            if last:
                # drain all deferred u-blocks first: PE has work while the
                # DVE runs the tail normalization chain
                while pending:
                    emit_u(*pending.pop(0))
            elif len(pending) > 1:
                emit_u(*pending.pop(0))

            # R = 1/S (custom DVE ~18-bit) then cast to bf16 for 2x muls
            r32 = r32pool.tile([128, NW], f32, tag="r32")
            nc.vector.reciprocal_approx_fast(r32, s_ps)
            r_bf = rpool.tile([128, NW], bf, tag="r")
            nc.vector.tensor_copy(r_bf, r32)

            r_base = r_bf[:]
            r_bc = bass.AP(
                tensor=r_base.tensor,
                offset=r_base.offset,
                ap=[r_base.ap[0], [0, B], [1, NW]],
            )

            g_t = gpool.tile([128, B, NW], bf, tag="g")
            gv_t = gvpool.tile([128, B, NW], bf, tag="gv")
            if not last:
                r2 = bass.AP(
                    tensor=r_base.tensor,
                    offset=r_base.offset,
                    ap=[r_base.ap[0], [0, 2], [1, NW]],
                )
                for bp in range(B // 2):
                    bsl = slice(2 * bp, 2 * bp + 2)
                    nc.vector.tensor_mul(g_t[:, bsl, :], e_t[:, bsl, :], r2)
                    nc.vector.tensor_mul(
                        gv_t[:, bsl, :], g_t[:, bsl, :], v_rep[:, pr, bsl, :]
                    )
                pending.append((gv_t, csl))
            else:
                # b-pair granular so the PE u-matmuls chase the DVE chain
                r2 = bass.AP(
                    tensor=r_base.tensor,
                    offset=r_base.offset,
                    ap=[r_base.ap[0], [0, 2], [1, NW]],
                )
                u_ps = upool.tile([32, NW], f32, tag="u")
                for bp in range(B // 2):
                    bsl = slice(2 * bp, 2 * bp + 2)
                    nc.vector.tensor_mul(g_t[:, bsl, :], e_t[:, bsl, :], r2)
                    nc.vector.tensor_mul(
                        gv_t[:, bsl, :], g_t[:, bsl, :], v_rep[:, pr, bsl, :]
                    )
                    for bb in range(2 * bp, 2 * bp + 2):
                        for half in range(2):
                            usl = slice(half * 512, (half + 1) * 512)
                            nc.tensor.matmul(
                                u_ps[:, usl],
                                L_sb[:, bb, :],
                                gv_t[:, bb, usl],
                                start=(bb == 0),
                                stop=(bb == B - 1),
                                skip_group_check=True,
                            )
                u_sb = usbpool.tile([32, NW], f32, tag="u_sb")
                nc.scalar.copy(u_sb, u_ps)
                nc.sync.dma_start(out=u_d[:, csl], in_=u_sb)

        for p in pending:
            emit_u(*p)

    nc.compile()
    nd, nw = _dedupe_ldweights(nc)
    print(f"ldweights dedupe: dropped {nd} (moved {nw} waits)")
    return nc


def _install_profile_hook():
    """The image's antenv lacks axon_hooks; inject it and register the
    ctypes NTFF hook from trn_agent_boot so trace=True works under axon."""
    import types

    try:
        from antenv.axon_hooks import get_axon_ntff_profile_hook  # noqa: F401

        return
    except ImportError:
        pass
    import antenv

    mod = types.ModuleType("antenv.axon_hooks")
    holder = {"hook": None}
    mod.set_axon_ntff_profile_hook = lambda h: holder.__setitem__("hook", h)
    mod.get_axon_ntff_profile_hook = lambda: holder["hook"]
    sys.modules["antenv.axon_hooks"] = mod
    antenv.axon_hooks = mod
    try:
        if "/root/.axon_site" not in sys.path:
            sys.path.insert(0, "/root/.axon_site")
        from trn_agent_boot.trn_boot import _ntff_profile_via_ctypes

        mod.set_axon_ntff_profile_hook(
            _ntff_profile_via_ctypes("/opt/axon/libaxon_pjrt.so")
        )
    except Exception as ex:  # degrade: tracing skipped, run still works
        print("profile hook install failed:", ex)
    # artifact upload needs bucket creds this container doesn't have
    import concourse.bass_utils as bu

    bu.upload_artifacts = lambda tmpdir: "local://" + tmpdir


_NC_CACHE = {}


def _get_nc():
    if "nc" not in _NC_CACHE:
        _NC_CACHE["nc"] = build_nc()
    return _NC_CACHE["nc"]


def make_inputs(h, v, W, U, b, w):
    """Host-side prep: shared tensors + per-core in_maps."""
    Wp = np.ascontiguousarray(
        W.reshape(DV // 128, 128, D).transpose(1, 0, 2)
    ).astype(BF16)
    vT = v.transpose(2, 0, 1).reshape(DV, B * SV)
    vTp = np.ascontiguousarray(
        vT.reshape(DV // 128, 128, B * SV).transpose(1, 0, 2)
    ).astype(BF16)
    w8p = np.ascontiguousarray(
        (w * W8SCALE).reshape(D // 128, 128, DV).transpose(1, 0, 2)
    ).astype(F8E4)
    vq = v.transpose(1, 0, 2).reshape(SV, B, NT, NW)  # (s, b, cq, cw)
    v3q = np.ascontiguousarray(
        np.tile(vq.transpose(2, 0, 1, 3), (1, SHL, 1, 1))
    ).astype(BF16)  # (cq, 128=(h,s), b, cw)
    U2 = np.concatenate([U, b[None, :]], axis=0).astype(F32)
    Ieye = np.eye(128, dtype=BF16)
    Lsum = np.zeros((128, B, ROWS), dtype=BF16)
    for bb in range(B):
        for hh in range(SHL):
            for ss in range(SV):
                Lsum[hh * SV + ss, bb, bb * SHL + hh] = 1
    in_maps = []
    for core in range(NCORES):
        hsl = h[:, core * SHL : (core + 1) * SHL, :]  # (B, SHL, DH)
        hT = np.ascontiguousarray(hsl.transpose(2, 0, 1).reshape(DH, ROWS))
        hT2 = np.concatenate([hT, np.ones((1, ROWS), F32)], axis=0).astype(F32)
        in_maps.append(
            {
                "Wp": Wp,
                "vTp": vTp,
                "w8p": w8p,
                "v3q": v3q,
                "U2": U2,
                "hT2": hT2,
                "Ieye": Ieye,
                "Lsump": Lsum,
            }
        )
    return in_maps


def gather_output(results):
    u_full = np.empty((B, SH, DV), dtype=F32)
    for core, res in enumerate(results):
        u_full[:, core * SHL : (core + 1) * SHL, :] = res["u_out"].reshape(
            B, SHL, DV
        )
    return u_full


def kernel(h, v, W, U, b, w, trace: bool = False):
    if trace:
        _install_profile_hook()
    nc = _get_nc()
    in_maps = make_inputs(
        np.asarray(h, F32),
        np.asarray(v, F32),
        np.asarray(W, F32),
        np.asarray(U, F32),
        np.asarray(b, F32),
        np.asarray(w, F32),
    )
    out = run_bass_kernel_spmd(nc, in_maps, core_ids=list(range(NCORES)), trace=trace)
    res = gather_output(out.results)
    if trace:
        kernel.last_exec_time_ns = out.exec_time_ns
        kernel.last_trace = out.instructions_and_trace
    return res
